# revision 21
# baseline (speedup 1.0000x reference)
"""Trainium2 Bass kernel for the DeformableSDFHead MLP.

Network (per point n, 16 bones k):
  x_k = [xyz3_k (3) | gl (48)]            gl shared per group of 4096 points
  h0  = relu(W0_k x_k + b0_k)             51 -> 64
  h_{l+1} = relu(Wmid_{k,l} h_l + bmid + h_l)   l = 0..6  (residual)
  latent = pre-residual out of l=6
  y = tanh(concat_k(latent_k) . Wf + bf)

Restructuring (all folds done host-side in numpy):
  * gl part of layer 0 folded into a per-(bone, group) bias beff.
  * residual folded into weights: W' = Wmid + I for l=0..5.
  * layer l=6 + final projection folded into a per-bone matvec:
      y = tanh(sum_k v_k . h6_k + c),  v_k = Wmid_{k,6}^T Wf_k.

Mapping: data-parallel over 8 cores (8192 points each). Per core, bones are
packed 4-at-a-time into the 128x128 PE array via tile_position (2x2 grid of
64x64 tiles), activations fp16, psum fp32, relu+bias evacuation split across
ScalarE (activation Relu w/ bias) and VectorE (tensor_scalar add+max).
"""

import numpy as np

import concourse.bacc as bacc
import concourse.bass as bass
import concourse.mybir as mybir
from concourse.tile import TileContext
from concourse.bass_utils import run_bass_kernel_spmd

NUM_BONES = 16
HID = 64
JOINT_IDX = np.array([0, 1, 2, 3, 5, 6, 7, 9, 10, 11, 13, 14, 15, 17, 18, 19])

NCORES = 8
N = 65536
NS = N // NCORES       # 8192 points per core
SG = 2048              # supergroup (points held in SBUF per pipeline stage)
NSG = NS // SG         # 4
F = 512                # matmul free-dim chunk (one psum bank)
RF = 1024              # relu op free dim (2 psum banks)

FP16 = mybir.dt.float16
FP32 = mybir.dt.float32

_SIGMA = [[(2 * p, 2 * p + 1) for p in range(8)]]
for _l in range(6):
    _SIGMA.append([_SIGMA[-1][p] if p % 2 == 0 else _SIGMA[-1][p][::-1]
                   for p in range(8)])


def _host_prep(xyz, joints, W0, b0, Wmid, bmid, Wf, bf):
    f32 = np.float32
    B = joints.shape[0]
    gl = joints[:, JOINT_IDX, :]
    gl = (gl - gl[:, :1, :]).reshape(B, -1).astype(f32)

    W0a = W0[:, :, 0:3].astype(f32)
    W0b = W0[:, :, 3:].astype(f32)
    beff = b0[:, None, :].astype(f32) + np.einsum('gi,koi->kgo', gl, W0b)

    I = np.eye(HID, dtype=f32)
    Wm_f = Wmid[:, :6].astype(f32) + I[None, None]

    Wf_k = Wf.reshape(NUM_BONES, HID).astype(f32)
    v = np.einsum('koi,ko->ki', Wmid[:, 6].astype(f32), Wf_k)
    c = float(np.sum(bmid[:, 6].astype(f32) * Wf_k) + bf[0])

    wm = np.zeros((128, 6 * 8 * 64), dtype=f32)
    bm = np.zeros((128, 48), dtype=f32)
    for l in range(6):
        for p in range(8):
            col = (l * 8 + p) * 64
            blo, bhi = _SIGMA[l][p]
            wm[0:64, col:col + 64] = Wm_f[blo, l].T
            wm[64:128, col:col + 64] = Wm_f[bhi, l].T
            olo, ohi = _SIGMA[l + 1][p]
            bm[0:64, l * 8 + p] = bmid[olo, l]
            bm[64:128, l * 8 + p] = bmid[ohi, l]

    w0 = np.zeros((128, 256), dtype=f32)
    for g in range(4):
        for j in range(4):
            w0[32 * j:32 * j + 3, 64 * g:64 * (g + 1)] = W0a[4 * g + j].T

    vt = np.zeros((128, 8 * 32), dtype=f32)
    for p in range(8):
        blo, bhi = _SIGMA[6][p]
        vt[0:64, 32 * p:32 * p + 32] = v[blo][:, None]
        vt[64:128, 32 * p:32 * p + 32] = v[bhi][:, None]

    xyzf = xyz.astype(f32)
    in_maps = []
    for core in range(NCORES):
        n0 = core * NS
        x3 = np.zeros((4, 12, NS), dtype=np.float16)
        for g in range(4):
            for j in range(4):
                b_ = 4 * g + j
                x3[g, 3 * j:3 * j + 3, :] = (
                    xyzf[n0:n0 + NS, 3 * (b_ + 1):3 * (b_ + 1) + 3].T.astype(np.float16))
        b0e = np.zeros((128, 16), dtype=f32)
        for p in range(8):
            blo, bhi = _SIGMA[0][p]
            for gi in range(2):
                grp = 2 * core + gi
                b0e[0:64, p * 2 + gi] = beff[blo, grp]
                b0e[64:128, p * 2 + gi] = beff[bhi, grp]
        in_maps.append(dict(
            x3=x3,
            w0=w0.astype(np.float16),
            wm=wm.astype(np.float16),
            bm=bm,
            b0e=b0e,
            vt=vt.astype(np.float16),
        ))
    return in_maps, c


_CACHE = {}


def _build():
    nc = bacc.Bacc("TRN2", target_bir_lowering=False)

    x3_h = nc.dram_tensor("x3", [4, 12, NS], FP16, kind="ExternalInput")
    w0_h = nc.dram_tensor("w0", [128, 256], FP16, kind="ExternalInput")
    wm_h = nc.dram_tensor("wm", [128, 6 * 8 * 64], FP16, kind="ExternalInput")
    bm_h = nc.dram_tensor("bm", [128, 48], FP32, kind="ExternalInput")
    b0e_h = nc.dram_tensor("b0e", [128, 16], FP32, kind="ExternalInput")
    vt_h = nc.dram_tensor("vt", [128, 8 * 32], FP16, kind="ExternalInput")
    out_h = nc.dram_tensor("out", [NSG, 4, F], FP32, kind="ExternalOutput")

    Relu = mybir.ActivationFunctionType.Relu
    Tanh = mybir.ActivationFunctionType.Tanh
    ADD = mybir.AluOpType.add
    MAX = mybir.AluOpType.max

    # relu engine split by measured rates (ACT 1114ns vs DVE 1283ns per op,
    # ACT also does tanh): ACT gets 17 of every 32.
    act_pick = [((i * 17) % 32) < 17 for i in range(32)]

    with TileContext(nc) as tc:
        with (
            tc.tile_pool(name="const", bufs=1) as cpool,
            tc.tile_pool(name="xin", bufs=2) as xpool,
            tc.tile_pool(name="hbuf", bufs=2) as hpool,
            tc.tile_pool(name="outp", bufs=2) as opool,
            tc.tile_pool(name="ps", bufs=4, space="PSUM") as pspool,
        ):
            w0_t = cpool.tile([128, 256], FP16, name="w0t")
            wm_t = cpool.tile([128, 6 * 8 * 64], FP16, name="wmt")
            bm_t = cpool.tile([128, 48], FP32, name="bmt")
            b0e_t = cpool.tile([128, 16], FP32, name="b0et")
            vt_t = cpool.tile([128, 8 * 32], FP16, name="vtt")
            nc.sync.dma_start(out=w0_t[:, :], in_=w0_h[:, :])
            nc.sync.dma_start(out=b0e_t[:, :], in_=b0e_h[:, :])
            nc.sync.dma_start(out=vt_t[:, :], in_=vt_h[:, :])

            relu_i = 0
            pending_mv = None

            def emit_matvec(h6, msg):
                # 4 col-concurrent accumulation chains (one per cc), p-outer
                # so the chains interleave and overlap in the PE array.
                mv = pspool.tile([128, RF], FP32, name="mv", tag="ps")[:, :F]
                for p in range(8):
                    for cc in range(4):
                        nc.tensor.matmul(
                            out=mv[32 * cc:32 * cc + 32, :],
                            lhsT=vt_t[:, 32 * p:32 * p + 32],
                            rhs=h6[p][:, cc * F:(cc + 1) * F],
                            start=(p == 0), stop=(p == 7),
                            tile_position=(0, 32 * cc),
                            skip_group_check=True)
                out_sb = opool.tile([128, F], FP32, name="osb", tag="osb")
                nc.vector.tensor_copy(out_sb[0:97, :], mv[0:97, :])
                ou_v = out_sb.rearrange("(a b) f -> a b f", b=32)[:, 0:1, :]
                nc.sync.dma_start(out=out_h[msg, :, :], in_=ou_v)

            def emit_relu(ps_ap, out_ap, bias_ap):
                nonlocal relu_i
                if act_pick[relu_i % 32]:
                    nc.scalar.activation(out_ap, ps_ap, Relu, bias=bias_ap, scale=1.0)
                else:
                    nc.vector.tensor_scalar(out_ap, ps_ap, bias_ap, 0.0, ADD, MAX)
                relu_i += 1

            for sg in range(NSG):
                s0 = sg * SG
                glocal = sg // 2

                xg = []
                for g in range(4):
                    xt = xpool.tile([128, SG], FP16, name=f"x{g}", tag=f"x{g}")
                    for j in range(4):
                        nc.sync.dma_start(
                            out=xt[32 * j:32 * j + 3, :],
                            in_=x3_h[g, 3 * j:3 * j + 3, s0:s0 + SG])
                    xg.append(xt)
                if sg == 0:
                    nc.sync.dma_start(out=bm_t[:, :], in_=bm_h[:, :])
                    nc.sync.dma_start(out=wm_t[:, :], in_=wm_h[:, :])

                # ---- layer 0 ----
                h_cur = [hpool.tile([128, SG], FP16, name=f"h{p}_a", tag=f"h{p}_a")
                         for p in range(8)]
                for g in range(4):
                    for half in range(2):
                        psA = pspool.tile([128, RF], FP32, name="psA", tag="ps")
                        psB = pspool.tile([128, RF], FP32, name="psB", tag="ps")
                        for ccl in range(2):
                            cc = 2 * half + ccl
                            for j in range(4):
                                ps = psA if j < 2 else psB
                                colh = 64 * (j % 2)
                                nc.tensor.matmul(
                                    out=ps[colh:colh + 64, ccl * F:(ccl + 1) * F],
                                    lhsT=w0_t[32 * j:32 * j + 3, 64 * g:64 * (g + 1)],
                                    rhs=xg[g][32 * j:32 * j + 3, cc * F:(cc + 1) * F],
                                    start=True, stop=True,
                                    tile_position=(32 * j, colh))
                        hs = slice(half * RF, (half + 1) * RF)
                        pA, pB = 2 * g, 2 * g + 1
                        emit_relu(psA[:, :], h_cur[pA][:, hs],
                                  b0e_t[:, pA * 2 + glocal:pA * 2 + glocal + 1])
                        emit_relu(psB[:, :], h_cur[pB][:, hs],
                                  b0e_t[:, pB * 2 + glocal:pB * 2 + glocal + 1])

                if pending_mv is not None:
                    emit_matvec(*pending_mv)
                    pending_mv = None

                # ---- mid layers l=0..5 ----
                for l in range(6):
                    suf = "b" if l % 2 == 0 else "a"
                    h_nxt = [hpool.tile([128, SG], FP16, name=f"h{p}_{suf}",
                                        tag=f"h{p}_{suf}") for p in range(8)]
                    for q in range(4):
                        colA = (l * 8 + 2 * q) * 64
                        colB = (l * 8 + 2 * q + 1) * 64
                        for half in range(2):
                            psA = pspool.tile([128, RF], FP32, name="psA", tag="ps")
                            psB = pspool.tile([128, RF], FP32, name="psB", tag="ps")
                            for ccl in range(2):
                                cc = 2 * half + ccl
                                fs = slice(cc * F, (cc + 1) * F)
                                os_ = slice(ccl * F, (ccl + 1) * F)
                                nc.tensor.matmul(
                                    out=psA[0:64, os_],
                                    lhsT=wm_t[0:64, colA:colA + 64],
                                    rhs=h_cur[2 * q][0:64, fs],
                                    start=True, stop=True)
                                nc.tensor.matmul(
                                    out=psA[64:128, os_],
                                    lhsT=wm_t[64:128, colA:colA + 64],
                                    rhs=h_cur[2 * q][64:128, fs],
                                    start=True, stop=True)
                                nc.tensor.matmul(
                                    out=psB[64:128, os_],
                                    lhsT=wm_t[0:64, colB:colB + 64],
                                    rhs=h_cur[2 * q + 1][0:64, fs],
                                    start=True, stop=True)
                                nc.tensor.matmul(
                                    out=psB[0:64, os_],
                                    lhsT=wm_t[64:128, colB:colB + 64],
                                    rhs=h_cur[2 * q + 1][64:128, fs],
                                    start=True, stop=True)
                            hs = slice(half * RF, (half + 1) * RF)
                            emit_relu(psA[:, :], h_nxt[2 * q][:, hs],
                                      bm_t[:, l * 8 + 2 * q:l * 8 + 2 * q + 1])
                            emit_relu(psB[:, :], h_nxt[2 * q + 1][:, hs],
                                      bm_t[:, l * 8 + 2 * q + 1:l * 8 + 2 * q + 2])
                    h_cur = h_nxt

                pending_mv = (h_cur, sg)

            emit_matvec(*pending_mv)
    nc.finalize()
    return nc


def kernel(xyz, joints, W0, b0, Wmid, bmid, Wf, bf):
    in_maps, c = _host_prep(xyz, joints, W0, b0, Wmid, bmid, Wf, bf)
    key = "nc"
    if key not in _CACHE:
        _CACHE[key] = _build()
    nc = _CACHE[key]
    res = run_bass_kernel_spmd(nc, in_maps, core_ids=list(range(NCORES)))
    s = np.concatenate([r["out"].reshape(-1) for r in res.results])
    return np.tanh(s + c).reshape(N, 1).astype(np.float32)
